# revision 1
# baseline (speedup 1.0000x reference)
"""Bayesian MLP MC-sample kernel for one TRN2 chip (8 NeuronCores).

Problem: out[s, b, o] for S=32 MC samples of a 3-layer MLP
  dims 256 -> 512 -> 512 -> 64, batch B=2048,
  w_s = z_w[s] * exp(w_log_std) + w_mean   (per-sample reparameterized weights)
  h1 = tanh(x @ w0_s + b0_s); h2 = tanh(h1 @ w1_s + b1_s); out = h2 @ w2_s + b2_s

Sharding: MC-sample axis across the 8 cores (4 samples/core); x and the
mean/log_std parameters are replicated. No cross-core communication.

On-chip layout: everything stays transposed (features on partitions,
batch on the free dim) so the matmul contraction is always the partition
dim and no transposes are needed on device:
  h^T[dout, B] = W^T x^T   via  matmul(psum, lhsT=w[k_chunk, dout_chunk],
                                       rhs=hprev^T[k_chunk, batch_slice])
The host passes x^T in and transposes the (S, 64, B) result back to
(S, B, 64) when gathering -- pure layout work, no FLOPs.

Per-core engine usage:
  PE:  4 samples x 112 matmuls (N=512, f32r -> 1 cycle/row)
  ACT: exp(log_std) once; per-sample tanh(psum + bias) eviction
  DVE: per-sample w = z * sigma + mean (two passes, in place)
  DMA: z shards + replicated params + x^T in, out^T back
"""

import numpy as np

import concourse.bass as bass
import concourse.mybir as mybir
import concourse.tile as tile
from concourse import bacc
from concourse import bass_utils

F32 = mybir.dt.float32
F32R = mybir.dt.float32r
# matmul compute dtype: f32r (TF32-like, 1 cycle/row at N>=256) gives
# 2.4e-4 end-to-end rel err vs 3.9e-3 for bf16 at only ~4% more runtime
MMDT = F32R
AF = mybir.ActivationFunctionType
ts = bass.ts

S = 32
B = 2048
DIMS = [256, 512, 512, 64]
NCORES = 8
SL = S // NCORES  # samples per core
NSLICE = 512      # moving-dim slice (max for fp32 matmul, = 1 PSUM bank)
NB = B // NSLICE

# knobs test.py may override before the first kernel() call
RUN_KWARGS: dict = {}
LAST_RESULT = None

_CACHE: dict = {}


def _build_nc():
    nc = bacc.Bacc("TRN2", target_bir_lowering=False)

    xT = nc.dram_tensor("xT", [DIMS[0], B], F32, kind="ExternalInput")
    w_mean, w_ls, b_mean, b_ls, z_w, z_b = [], [], [], [], [], []
    for li in range(3):
        din, dout = DIMS[li], DIMS[li + 1]
        w_mean.append(nc.dram_tensor(f"w_mean_{li}", [din, dout], F32, kind="ExternalInput"))
        w_ls.append(nc.dram_tensor(f"w_log_std_{li}", [din, dout], F32, kind="ExternalInput"))
        b_mean.append(nc.dram_tensor(f"b_mean_{li}", [dout], F32, kind="ExternalInput"))
        b_ls.append(nc.dram_tensor(f"b_log_std_{li}", [dout], F32, kind="ExternalInput"))
        z_w.append(nc.dram_tensor(f"z_w_{li}", [SL, din, dout], F32, kind="ExternalInput"))
        z_b.append(nc.dram_tensor(f"z_b_{li}", [SL, dout], F32, kind="ExternalInput"))
    out_d = nc.dram_tensor("out", [SL, DIMS[3], B], F32, kind="ExternalOutput")

    NK = [d // 128 for d in DIMS[:3]]      # k-chunks per layer: 2, 4, 4
    MP = [min(128, d) for d in DIMS[1:]]   # psum partitions:  128, 128, 64
    NM = [d // 128 if d >= 128 else 1 for d in DIMS[1:]]  # m-chunks: 4, 4, 1
    BP = [min(128, d) for d in DIMS[1:]]   # bias partitions
    BC = [max(1, d // 128) for d in DIMS[1:]]  # bias cols

    with tile.TileContext(nc) as tc:
        with (
            tc.tile_pool(name="const", bufs=1) as cpool,
            tc.tile_pool(name="z", bufs=2) as zpool,
            tc.tile_pool(name="w0", bufs=2) as w0p,
            tc.tile_pool(name="w1", bufs=2) as w1p,
            tc.tile_pool(name="w2", bufs=2) as w2p,
            tc.tile_pool(name="h1", bufs=2) as h1p,
            tc.tile_pool(name="h2", bufs=1) as h2p,
            tc.tile_pool(name="osb", bufs=1) as opool,
            tc.tile_pool(name="ps", bufs=2, space="PSUM") as pspool,
        ):
            wpools = [w0p, w1p, w2p]
            # All transfers ride the sync-engine HWDGE ring: a large
            # dma_start spreads over all 16 SDMA engines (~350 GB/s), and
            # the ring serves transfers in issue order, which doubles as
            # the prefetch priority. (Measured: gpsimd SWDGE ~20 GB/s and
            # scalar-ring issues stall the ACT FIFO -- both hurt; small
            # transfers cost ~1.5-2 us of ring dead time each, hence the
            # per-layer batching of bias vectors.)
            hw1 = nc.sync
            sw = nc.sync

            sigma = [None] * 3
            mean = [None] * 3
            sigma_b = [None] * 3
            mean_b = [None] * 3

            ball = [None] * 3

            def emit_bias_layer(li, dma=None):
                # all SL samples' bias noise in ONE transfer -- small DMAs
                # cost ~1.5 us of ring dead time each
                dma = dma or sw
                bp, bc = BP[li], BC[li]
                bz = cpool.tile([bp, SL, bc], F32, tag=f"ball{li}")
                dma.dma_start(bz[:], z_b[li][:].rearrange("s (c p) -> p s c", p=bp))
                ball[li] = bz
                sgb = cpool.tile([bp, bc], F32, tag=f"sigma_b{li}")
                dma.dma_start(sgb[:], b_ls[li][:].rearrange("(c p) -> p c", p=bp))
                nc.scalar.activation(sgb[:], sgb[:], AF.Exp)
                sigma_b[li] = sgb
                mnb = cpool.tile([bp, bc], F32, tag=f"mean_b{li}")
                dma.dma_start(mnb[:], b_mean[li][:].rearrange("(c p) -> p c", p=bp))
                mean_b[li] = mnb

            def emit_consts(li):
                nk, dout = NK[li], DIMS[li + 1]
                sg = cpool.tile([128, nk, dout], F32, tag=f"sigma{li}")
                hw1.dma_start(sg[:], w_ls[li][:].rearrange("(k p) d -> p k d", p=128))
                nc.scalar.activation(sg[:], sg[:], AF.Exp)
                sigma[li] = sg
                mn = cpool.tile([128, nk, dout], F32, tag=f"mean{li}")
                hw1.dma_start(mn[:], w_mean[li][:].rearrange("(k p) d -> p k d", p=128))
                mean[li] = mn

            # per-sample state
            h_tiles = [dict(), dict()]
            w_tiles = dict()
            b_tiles = dict()

            def emit_bias(li, s):
                bsl = ball[li][:, s, :]
                nc.vector.tensor_mul(bsl, bsl, sigma_b[li][:])
                nc.vector.tensor_add(bsl, bsl, mean_b[li][:])
                b_tiles[(li, s)] = bsl

            def emit_wprep(li, s, dve_chunked=False, bias=True):
                nk, dout = NK[li], DIMS[li + 1]
                # sampled weights: w = z * sigma + mean (mul in place on the
                # z staging tile; the add writes the f32r weight tile -- the
                # BIR verifier requires every writer of a matmul operand's
                # memory location to produce rounded f32r)
                zt = zpool.tile([128, nk, dout], F32, tag="z")
                wt = wpools[li].tile([128, nk, dout], MMDT, tag=f"w{li}")
                hw1.dma_start(zt[:], z_w[li][s].rearrange("(k p) d -> p k d", p=128))
                ks = range(nk) if dve_chunked else [slice(None)]
                for k in ks:
                    nc.vector.tensor_mul(zt[:, k, :], zt[:, k, :], sigma[li][:, k, :])
                    nc.vector.tensor_add(wt[:, k, :], zt[:, k, :], mean[li][:, k, :])
                w_tiles[(li, s)] = wt
                if bias:
                    emit_bias(li, s)

            def get_dst(li, s):
                hp = h1p if li == 0 else h2p
                dst = hp.tile([128, NM[li], B], MMDT, tag=f"h{li}")
                h_tiles[li][s] = dst
                return dst

            def emit_matmuls(li, s, korder=False, split_last=False):
                nk, nm, mp = NK[li], NM[li], MP[li]
                wt = w_tiles.pop((li, s))
                bt = b_tiles.pop((li, s))
                src = xT_t if li == 0 else h_tiles[li - 1][s]
                if li < 2:
                    dst = get_dst(li, s)

                for m in range(nm):
                    ps = pspool.tile([mp, B], F32, tag="ps")
                    kn = (
                        [(k, n) for k in range(nk) for n in range(NB)]
                        if korder
                        else [(k, n) for n in range(NB) for k in range(nk)]
                    )
                    for k, n in kn:
                        nc.tensor.matmul(
                            ps[:, ts(n, NSLICE)],
                            wt[:, k, ts(m, mp)],
                            src[:, k, ts(n, NSLICE)],
                            start=(k == 0),
                            stop=(k == nk - 1),
                        )
                    # bank-wise eviction on boundary tiles: each PSUM bank is
                    # released right after its accumulation, so the consumer
                    # (subtile deps) starts ~3 banks earlier
                    split = split_last and m == nm - 1 and not korder
                    nslices = (
                        [ts(n, NSLICE) for n in range(NB)] if split else [slice(None)]
                    )
                    if li < 2:
                        for sl in nslices:
                            nc.scalar.activation(
                                dst[:, m, sl], ps[:, sl], AF.Tanh, bias=bt[:, m : m + 1]
                            )
                    else:
                        # output eviction on DVE (ACT is the second-busiest
                        # engine; DVE has slack)
                        osb = opool.tile([mp, B], F32, tag="osb")
                        odma = nc.scalar if s == SL - 1 else hw1
                        for sl in nslices:
                            nc.vector.tensor_scalar_add(osb[:, sl], ps[:, sl], bt[:, 0:1])
                            odma.dma_start(out_d[s][:, sl], osb[:, sl])
                        h_tiles[0].pop(s, None)
                        h_tiles[1].pop(s, None)

            # ---- PE warm-up ----
            # The PE clock is HAM-gated to 1.2 GHz until ~3.4us of sustained
            # activity. The PE would otherwise idle from its preamble (~5.5us)
            # until the first real matmul (~17us) and then run the first ~60
            # matmuls cold (427ns vs 213ns at N=512). Dummy bf16 matmuls on
            # zeroed scratch tiles warm the clock during the DMA-bound window.
            warm_w = cpool.tile([128, 128], mybir.dt.bfloat16, tag="warm_w")
            warm_x = cpool.tile([128, NSLICE], mybir.dt.bfloat16, tag="warm_x")
            nc.gpsimd.memset(warm_w[:], 0.0)
            nc.gpsimd.memset(warm_x[:], 0.0)
            # dummies share the first real psum tile (its first real matmul
            # has start=True, which resets it) so they cost no PSUM slot
            warm_ps = pspool.tile([128, NSLICE], F32, tag="ps")
            for _ in range(20):
                nc.tensor.matmul(warm_ps[:], warm_w[:], warm_x[:], start=True, stop=True)

            # ---- startup: minimal critical path for layer-0 sample-0 ----
            # sync-ring order = arrival priority: ls0, z0(0), mn0, x^T
            # quarters, then sample-1 z0, then the layer-1 inputs.
            sg0 = cpool.tile([128, NK[0], DIMS[1]], F32, tag="sigma0")
            hw1.dma_start(sg0[:], w_ls[0][:].rearrange("(k p) d -> p k d", p=128))
            nc.scalar.activation(sg0[:], sg0[:], AF.Exp)
            sigma[0] = sg0
            zt0 = zpool.tile([128, NK[0], DIMS[1]], F32, tag="z")
            hw1.dma_start(zt0[:], z_w[0][0].rearrange("(k p) d -> p k d", p=128))
            mn0 = cpool.tile([128, NK[0], DIMS[1]], F32, tag="mean0")
            hw1.dma_start(mn0[:], w_mean[0][:].rearrange("(k p) d -> p k d", p=128))
            mean[0] = mn0
            # layer-0 bias items ride gpsimd SWDGE: tiny (12 KB), needed
            # early, and keeping them out of the sync ring avoids ring-full
            # back-pressure ahead of the x^T quarter transfers
            emit_bias_layer(0, dma=nc.gpsimd)
            nc.vector.tensor_mul(zt0[:], zt0[:], sg0[:])
            wt0 = wpools[0].tile([128, NK[0], DIMS[1]], MMDT, tag="w0")
            nc.vector.tensor_add(wt0[:], zt0[:], mn0[:])
            emit_bias(0, 0)

            # x^T in quarter slices (f32 staging borrows an h1 slot), cast
            # slice-wise to f32r
            xT_stage = h1p.tile([128, NK[0], B], F32, tag="h0")
            xT_t = cpool.tile([128, NK[0], B], MMDT, tag="xT")
            xT_src = xT[:].rearrange("(k p) n -> p k n", p=128)
            for n in range(NB):
                hw1.dma_start(
                    xT_stage[:, :, ts(n, NSLICE)], xT_src[:, :, ts(n, NSLICE)]
                )
                nc.vector.tensor_copy(
                    xT_t[:, :, ts(n, NSLICE)], xT_stage[:, :, ts(n, NSLICE)]
                )

            # layer-0 sample-0: batch-major single-bank tiles so matmuls
            # start as soon as the first x^T quarter is cast
            dst00 = get_dst(0, 0)
            bt00 = b_tiles.pop((0, 0))
            for n in range(NB):
                for m in range(NM[0]):
                    if n == 0 and m == 0:
                        ps = warm_ps
                    else:
                        ps = pspool.tile([MP[0], NSLICE], F32, tag="ps")
                    for k in range(NK[0]):
                        nc.tensor.matmul(
                            ps[:],
                            wt0[:, k, ts(m, MP[0])],
                            xT_t[:, k, ts(n, NSLICE)],
                            start=(k == 0),
                            stop=(k == NK[0] - 1),
                        )
                    nc.scalar.activation(
                        dst00[:, m, ts(n, NSLICE)], ps[:], AF.Tanh,
                        bias=bt00[:, m : m + 1],
                    )
            # two-sample lookahead: sample-1 layer-0 covers the window while
            # the (3 MB) layer-1 inputs stream in
            emit_wprep(0, 1)
            emit_matmuls(0, 1)
            # layer-1 weights: sigma, then the z shard, then mean in
            # k-chunks interleaved with the DVE prep -- the first L1 matmul
            # (k-outer) then waits only on mean chunk 0, not the whole 1 MB
            sg1 = cpool.tile([128, NK[1], DIMS[2]], F32, tag="sigma1")
            hw1.dma_start(sg1[:], w_ls[1][:].rearrange("(k p) d -> p k d", p=128))
            nc.scalar.activation(sg1[:], sg1[:], AF.Exp)
            sigma[1] = sg1
            zt1 = zpool.tile([128, NK[1], DIMS[2]], F32, tag="z")
            wt1 = wpools[1].tile([128, NK[1], DIMS[2]], MMDT, tag="w1")
            hw1.dma_start(zt1[:], z_w[1][0].rearrange("(k p) d -> p k d", p=128))
            mn1 = cpool.tile([128, NK[1], DIMS[2]], F32, tag="mean1")
            mean[1] = mn1
            mn1_src = w_mean[1][:].rearrange("(k p) d -> p k d", p=128)
            for k in range(NK[1]):
                hw1.dma_start(mn1[:, k, :], mn1_src[:, k, :])
                nc.vector.tensor_mul(zt1[:, k, :], zt1[:, k, :], sg1[:, k, :])
                nc.vector.tensor_add(wt1[:, k, :], zt1[:, k, :], mn1[:, k, :])
            w_tiles[(1, 0)] = wt1
            emit_bias_layer(1)
            emit_bias(1, 0)
            emit_matmuls(1, 0, korder=True)
            emit_consts(2)
            emit_bias_layer(2)

            # steady state; split_last covers the only uncovered boundaries
            sched = [
                (0, 2, {}),
                (2, 0, {}),
                (1, 1, {}),
                (0, 3, {}),
                (2, 1, {}),
                (1, 2, dict(split_last=True)),
                (2, 2, {}),
                (1, 3, dict(split_last=True)),
                (2, 3, dict(split_last=True)),
            ]
            for li, s, kw in sched:
                emit_wprep(li, s)
                emit_matmuls(li, s, **kw)

    nc.compile()
    return nc


def _get_nc():
    if "nc" not in _CACHE:
        _CACHE["nc"] = _build_nc()
    return _CACHE["nc"]


def kernel(**inputs) -> np.ndarray:
    global LAST_RESULT
    nc = _get_nc()
    inp = {k: np.asarray(v, dtype=np.float32) for k, v in inputs.items()}

    xT = np.ascontiguousarray(inp["x"].T)
    in_maps = []
    for c in range(NCORES):
        sl = slice(c * SL, (c + 1) * SL)
        m = {"xT": xT}
        for li in range(3):
            m[f"w_mean_{li}"] = inp[f"w_mean_{li}"]
            m[f"w_log_std_{li}"] = inp[f"w_log_std_{li}"]
            m[f"b_mean_{li}"] = inp[f"b_mean_{li}"]
            m[f"b_log_std_{li}"] = inp[f"b_log_std_{li}"]
            m[f"z_w_{li}"] = np.ascontiguousarray(inp[f"z_w_{li}"][sl])
            m[f"z_b_{li}"] = np.ascontiguousarray(inp[f"z_b_{li}"][sl, 0, :])
        in_maps.append(m)

    res = bass_utils.run_bass_kernel_spmd(
        nc, in_maps, core_ids=list(range(NCORES)), **RUN_KWARGS
    )
    LAST_RESULT = res
    full = np.concatenate([res.results[c]["out"] for c in range(NCORES)], axis=0)
    return np.ascontiguousarray(full.transpose(0, 2, 1)).astype(np.float32)



# revision 5
# speedup vs baseline: 1.1265x; 1.1265x over previous
"""Bayesian MLP MC-sample kernel for one TRN2 chip (8 NeuronCores) - v2.

Problem: out[s, b, o] for S=32 MC samples of a 3-layer MLP
  dims 256 -> 512 -> 512 -> 64, batch B=2048,
  w_s = z_w[s] * exp(w_log_std) + w_mean   (per-sample reparameterized weights)
  h1 = tanh(x @ w0_s + b0_s); h2 = tanh(h1 @ w1_s + b1_s); out = h2 @ w2_s + b2_s

Sharding: MC-sample axis across the 8 cores (4 samples/core); x and the
mean/log_std parameters are replicated. No cross-core communication.

v2 design notes (from the v1 trace, 143.4us):
- All matmul operands bf16 (rel err ~4e-3 << 2e-2 gate). Same PE rate as
  f32r (1 col/cycle) but halves SBUF for h tiles, enabling deeper
  prefetch and 3 live h1 tiles.
- PSUM at bank granularity: [128,1024] 2-bank tiles, 3 rotating + warm
  + tail = 8 banks. Eviction per 2-bank tile on ACT (tanh+bias fused;
  L2 eviction is Copy+bias on ACT, out-DMA issued from the DVE queue so
  the sync ring never carries outputs).
- Startup: the v1 kernel's first real matmul was at 21.3us because the
  2.5MB of layer-0 params + x had to land first (sync ring ~290GB/s,
  first payload ~10.8us after the fixed ~8.5us engine preamble).
  v2 m-half-chunks the layer-0 params (dout 0:256 / 256:512) and
  interleaves x quarters so real MMs start ~16us, in DMA-arrival order.
- DMA order on the sync ring == consumption order; z transfers are
  half/k-chunked so DVE w-prep (w = z*sigma + mu) pipelines per chunk.
- Sample schedule: L0s0 L0s1 L0s2 L1s0 L0s3 L1s1 L2(s0,s1) L1s2 L1s3
  L2(s2,s3). Layer-2 pairs two samples' M=64 matmuls onto PE column
  strips via tile_position (0,0)/(0,64) - concurrent, halving L2 time.
- Bias data (z_b, b_mean, b_log_std: ~26KB) is host-packed into one
  [128, 52] f32 tensor in the exact SBUF layout (pure layout work);
  v1 spent a 7.6us DMA_DIRECT2D issue on a 4-byte-strided z_b pattern.
- ~34 bf16 warmup matmuls on zeroed tiles keep the PE HAM clock-gate
  warm (K=8/8, 2.4GHz) through the DMA-bound startup window.
"""

import numpy as np

import concourse.bass as bass
import concourse.mybir as mybir
import concourse.tile as tile
from concourse import bacc
from concourse import bass_utils

F32 = mybir.dt.float32
BF16 = mybir.dt.bfloat16
MMDT = BF16
AF = mybir.ActivationFunctionType
ts = bass.ts

S = 32
B = 2048
DIMS = [256, 512, 512, 64]
NCORES = 8
SL = S // NCORES   # samples per core
NS = 512           # one PSUM bank of f32
NB = B // NS       # 4 n-slices
NK = [d // 128 for d in DIMS[:3]]        # k-chunks per layer: 2, 4, 4
NM = [max(1, d // 128) for d in DIMS[1:]]  # m-chunks: 4, 4, 1

# host-packed bias tensor layout: [128, BP_W] f32
# per layer: exp-able b_log_std block, b_mean block, z_b blocks
BC = [4, 4, 1]          # cols per item (dout/128; L2 uses 64 partitions)
BLS = [0, 24, 48]       # b_log_std col offset
BMN = [4, 28, 49]       # b_mean col offset
BZB = [8, 32, 50]       # z_b col offset (L0/L1: BC per sample; L2: 1 col/pair)
BP_W = 52

# knobs test.py may override before the first kernel() call
RUN_KWARGS: dict = {}
LAST_RESULT = None

_CACHE: dict = {}


def _build_nc():
    nc = bacc.Bacc("TRN2", target_bir_lowering=False)

    xT = nc.dram_tensor("xT", [DIMS[0], B], F32, kind="ExternalInput")
    biaspack = nc.dram_tensor("biaspack", [128, BP_W], F32, kind="ExternalInput")
    w_mean, w_ls, z_w = [], [], []
    for li in range(3):
        din, dout = DIMS[li], DIMS[li + 1]
        w_mean.append(nc.dram_tensor(f"w_mean_{li}", [din, dout], F32, kind="ExternalInput"))
        w_ls.append(nc.dram_tensor(f"w_log_std_{li}", [din, dout], F32, kind="ExternalInput"))
        z_w.append(nc.dram_tensor(f"z_w_{li}", [SL, din, dout], F32, kind="ExternalInput"))
    out_d = nc.dram_tensor("out", [SL, DIMS[3], B], F32, kind="ExternalOutput")

    with tile.TileContext(nc) as tc:
        with (
            tc.tile_pool(name="const", bufs=1) as cpool,
            tc.tile_pool(name="z", bufs=4) as zpool,
            tc.tile_pool(name="w0", bufs=2) as w0p,
            tc.tile_pool(name="w1", bufs=2) as w1p,
            tc.tile_pool(name="w2", bufs=2) as w2p,
            tc.tile_pool(name="h1", bufs=3) as h1p,
            tc.tile_pool(name="h2", bufs=2) as h2p,
            tc.tile_pool(name="osb", bufs=2) as opool,
            tc.tile_pool(name="ps", bufs=3, space="PSUM") as pspool,
            tc.tile_pool(name="psw", bufs=1, space="PSUM") as pswp,
            tc.tile_pool(name="pst", bufs=1, space="PSUM") as pstp,
        ):
            hwd = nc.sync      # input DMA ring (issue order == priority)
            # output DMAs are issued from the ACT queue right after each
            # eviction (DVE can't issue DMAs; sync would head-of-line-block
            # the later-emitted z prefetches behind the ~78us L2 evictions)
            odma = nc.scalar

            sigma = [None] * 3   # exp(w_log_std), f32, [128, nk, dout]
            mean = [None] * 3    # w_mean, f32, [128, nk, dout]
            w_tiles = {}
            h1_tiles = {}
            h2_tiles = {}

            # ---- bias pack ----
            bp_t = cpool.tile([128, BP_W], F32, tag="bp")

            def bias_ap(li, s):
                c = BC[li]
                return bp_t[:, BZB[li] + c * s : BZB[li] + c * (s + 1)]

            def emit_bias_exp(li):
                c = BC[li]
                sl_ = bp_t[:, BLS[li] : BLS[li] + c]
                nc.scalar.activation(sl_, sl_, AF.Exp)

            def emit_bias_prep(li, s):
                c = BC[li]
                col = bias_ap(li, s)
                nc.vector.tensor_mul(col, col, bp_t[:, BLS[li] : BLS[li] + c])
                nc.vector.tensor_add(col, col, bp_t[:, BMN[li] : BMN[li] + c])

            # ---- generic w-prep: w = z * sigma + mean, chunked DVE ----
            def emit_wprep(li, s, wpool, half_dma=False):
                nk, dout = NK[li], DIMS[li + 1]
                zt = zpool.tile([128, nk, dout], F32, tag="z")
                wt = wpool.tile([128, nk, dout], MMDT, tag=f"w{li}")
                zsrc = z_w[li][s].rearrange("(k p) d -> p k d", p=128)
                if half_dma and nk >= 2:
                    h = nk // 2
                    hwd.dma_start(zt[:, 0:h, :], zsrc[:, 0:h, :])
                    hwd.dma_start(zt[:, h:nk, :], zsrc[:, h:nk, :])
                else:
                    hwd.dma_start(zt[:], zsrc)
                for k in range(nk):
                    nc.vector.tensor_mul(zt[:, k, :], zt[:, k, :], sigma[li][:, k, :])
                    nc.vector.tensor_add(wt[:, k, :], zt[:, k, :], mean[li][:, k, :])
                w_tiles[(li, s)] = wt
                if li < 2:
                    emit_bias_prep(li, s)

            # ---- layer 0/1 matmuls: per-m, per-n-pair 2-bank psum tiles ----
            def emit_l01_mms(li, s, korder=False):
                nk = NK[li]
                wt = w_tiles.pop((li, s))
                bt = bias_ap(li, s)
                src = xbf if li == 0 else h1_tiles[s]
                if li == 0:
                    dst = h1p.tile([128, NM[0], B], MMDT, tag="h1")
                    h1_tiles[s] = dst
                else:
                    dst = h2p.tile([128, NM[1], B], MMDT, tag="h2")
                    h2_tiles[s] = dst
                for m in range(NM[li]):
                    if korder:
                        # first bank starts as soon as w k-chunk 0 is ready
                        pss = [
                            pspool.tile([128, 2 * NS], F32, tag="ps", name=f"ps_ko{i}")
                            for i in range(2)
                        ]
                        for k in range(nk):
                            for npair in range(2):
                                for nn in range(2):
                                    n = npair * 2 + nn
                                    nc.tensor.matmul(
                                        pss[npair][:, ts(nn, NS)],
                                        wt[:, k, ts(m, 128)],
                                        src[:, k, ts(n, NS)],
                                        start=(k == 0),
                                        stop=(k == nk - 1),
                                    )
                        for npair in range(2):
                            nc.scalar.activation(
                                dst[:, m, ts(npair, 2 * NS)], pss[npair][:],
                                AF.Tanh, bias=bt[:, m : m + 1],
                            )
                    else:
                        for npair in range(2):
                            ps = pspool.tile([128, 2 * NS], F32, tag="ps")
                            for nn in range(2):
                                n = npair * 2 + nn
                                for k in range(nk):
                                    nc.tensor.matmul(
                                        ps[:, ts(nn, NS)],
                                        wt[:, k, ts(m, 128)],
                                        src[:, k, ts(n, NS)],
                                        start=(k == 0),
                                        stop=(k == nk - 1),
                                    )
                            nc.scalar.activation(
                                dst[:, m, ts(npair, 2 * NS)], ps[:],
                                AF.Tanh, bias=bt[:, m : m + 1],
                            )
                if li == 1:
                    h1_tiles.pop(s, None)

            # ---- layer 2: two samples packed on PE column strips ----
            def emit_l2_pair(j, tail=False):
                sa, sb = 2 * j, 2 * j + 1
                wa = w_tiles.pop((2, sa))
                wb = w_tiles.pop((2, sb))
                ha = h2_tiles.pop(sa)
                hb = h2_tiles.pop(sb)
                bt = bias_ap(2, j)  # [128,1]: sa bias on parts 0-63, sb on 64-127
                nk = NK[2]

                def strip_mms(psl, n):
                    for k in range(nk):
                        nc.tensor.matmul(
                            psl[0:64, ts(n % 2, NS)], wa[:, k, :], ha[:, k, ts(n, NS)],
                            start=(k == 0), stop=(k == nk - 1), tile_position=(0, 0),
                        )
                        nc.tensor.matmul(
                            psl[64:128, ts(n % 2, NS)], wb[:, k, :], hb[:, k, ts(n, NS)],
                            start=(k == 0), stop=(k == nk - 1), tile_position=(0, 64),
                        )

                for npair in range(2):
                    if tail and npair == 1:
                        # finer tail: per-bank psum, per-sample eviction + DMA
                        for nn in range(2):
                            n = 2 + nn
                            pt = pstp.tile([128, NS], F32, tag="pst")
                            for k in range(nk):
                                nc.tensor.matmul(
                                    pt[0:64, :], wa[:, k, :], ha[:, k, ts(n, NS)],
                                    start=(k == 0), stop=(k == nk - 1), tile_position=(0, 0),
                                )
                                nc.tensor.matmul(
                                    pt[64:128, :], wb[:, k, :], hb[:, k, ts(n, NS)],
                                    start=(k == 0), stop=(k == nk - 1), tile_position=(0, 64),
                                )
                            osb = opool.tile([128, 2 * NS], F32, tag="osb")
                            nc.scalar.activation(
                                osb[:, ts(nn, NS)], pt[:], AF.Identity, bias=bt[:, 0:1]
                            )
                            odma.dma_start(out_d[sa][:, ts(n, NS)], osb[0:64, ts(nn, NS)])
                            odma.dma_start(out_d[sb][:, ts(n, NS)], osb[64:128, ts(nn, NS)])
                    else:
                        ps = pspool.tile([128, 2 * NS], F32, tag="ps")
                        strip_mms(ps, npair * 2)
                        strip_mms(ps, npair * 2 + 1)
                        osb = opool.tile([128, 2 * NS], F32, tag="osb")
                        nc.scalar.activation(osb[:], ps[:], AF.Identity, bias=bt[:, 0:1])
                        odma.dma_start(out_d[sa][:, ts(npair, 2 * NS)], osb[0:64, :])
                        odma.dma_start(out_d[sb][:, ts(npair, 2 * NS)], osb[64:128, :])

            # ================= PE warm-up =================
            # HAM gates the PE clock to 1.2GHz until ~3.4us of sustained
            # activity; dummy bf16 matmuls cover the DMA-bound startup.
            warm_w = cpool.tile([128, 128], BF16, tag="warm_w")
            warm_x = cpool.tile([128, NS], BF16, tag="warm_x")
            nc.gpsimd.memset(warm_w[:], 0.0)
            nc.gpsimd.memset(warm_x[:], 0.0)
            warm_ps = pswp.tile([128, NS], F32, tag="psw")
            for _ in range(34):
                nc.tensor.matmul(warm_ps[:], warm_w[:], warm_x[:], start=True, stop=True)

            # ================= startup: layer-0 sample-0, m-half chunked =====
            # sync ring order (== arrival order):
            #   s0A(ls,z,mn) xq0 bias xq1 s0B xq2 xq3 z01 z02 s1(ls,z,mn) z03
            #   z11 s2consts z20 z21 z12 z13 z22 z23
            HD = 256  # dout half for layer-0 param chunks
            sg0 = cpool.tile([128, NK[0], DIMS[1]], F32, tag="sigma0")
            zt0 = zpool.tile([128, NK[0], DIMS[1]], F32, tag="z")
            mn0 = cpool.tile([128, NK[0], DIMS[1]], F32, tag="mean0")
            wt0 = w0p.tile([128, NK[0], DIMS[1]], MMDT, tag="w0")
            sigma[0], mean[0] = sg0, mn0
            sg0_src = w_ls[0][:].rearrange("(k p) d -> p k d", p=128)
            z0_src = z_w[0][0].rearrange("(k p) d -> p k d", p=128)
            mn0_src = w_mean[0][:].rearrange("(k p) d -> p k d", p=128)
            xbf = cpool.tile([128, NK[0], B], MMDT, tag="xbf")
            x_stage = cpool.tile([128, NK[0], B], F32, tag="xstg")
            x_src = xT[:].rearrange("(k p) n -> p k n", p=128)

            def l0_param_half(h):
                d0 = ts(h, HD)
                hwd.dma_start(sg0[:, :, d0], sg0_src[:, :, d0])
                nc.scalar.activation(sg0[:, :, d0], sg0[:, :, d0], AF.Exp)
                hwd.dma_start(zt0[:, :, d0], z0_src[:, :, d0])
                hwd.dma_start(mn0[:, :, d0], mn0_src[:, :, d0])
                nc.vector.tensor_mul(zt0[:, :, d0], zt0[:, :, d0], sg0[:, :, d0])
                nc.vector.tensor_add(wt0[:, :, d0], zt0[:, :, d0], mn0[:, :, d0])

            def x_quarter(q):
                hwd.dma_start(x_stage[:, :, ts(q, NS)], x_src[:, :, ts(q, NS)])
                nc.scalar.activation(
                    xbf[:, :, ts(q, NS)], x_stage[:, :, ts(q, NS)], AF.Copy
                )

            l0_param_half(0)
            x_quarter(0)
            hwd.dma_start(bp_t[:], biaspack[:])
            for li in range(3):
                emit_bias_exp(li)
            emit_bias_prep(0, 0)
            x_quarter(1)
            l0_param_half(1)
            x_quarter(2)
            x_quarter(3)

            # L0 s0 matmuls in DMA-arrival order; [128,1024] tiles hold two
            # (m, n) banks, each evicted separately (different m -> different
            # h1 slice).
            dst00 = h1p.tile([128, NM[0], B], MMDT, tag="h1")
            h1_tiles[0] = dst00
            bt00 = bias_ap(0, 0)
            groups = [
                (0, 0), (1, 0), (0, 1), (1, 1),  # half A, q0/q1
                (2, 0), (3, 0), (2, 1), (3, 1),  # half B
                (0, 2), (1, 2), (0, 3), (1, 3),  # half A, q2/q3
                (2, 2), (3, 2), (2, 3), (3, 3),  # half B
            ]
            for gi in range(0, len(groups), 2):
                ps = pspool.tile([128, 2 * NS], F32, tag="ps")
                for half, (m, n) in enumerate(groups[gi : gi + 2]):
                    for k in range(NK[0]):
                        nc.tensor.matmul(
                            ps[:, ts(half, NS)],
                            wt0[:, k, ts(m, 128)],
                            xbf[:, k, ts(n, NS)],
                            start=(k == 0),
                            stop=(k == NK[0] - 1),
                        )
                    nc.scalar.activation(
                        dst00[:, m, ts(n, NS)], ps[:, ts(half, NS)],
                        AF.Tanh, bias=bt00[:, m : m + 1],
                    )
            w_tiles[(0, 0)] = None  # consumed above

            # ---- L0 s1, s2 (cover the layer-1 param DMA window) ----
            emit_wprep(0, 1, w0p, half_dma=True)
            emit_l01_mms(0, 1)
            emit_wprep(0, 2, w0p, half_dma=True)

            # layer-1 consts, k-chunked so DVE prep pipelines per chunk
            sg1 = cpool.tile([128, NK[1], DIMS[2]], F32, tag="sigma1")
            zt1 = zpool.tile([128, NK[1], DIMS[2]], F32, tag="z")
            mn1 = cpool.tile([128, NK[1], DIMS[2]], F32, tag="mean1")
            wt1 = w1p.tile([128, NK[1], DIMS[2]], MMDT, tag="w1")
            sigma[1], mean[1] = sg1, mn1
            sg1_src = w_ls[1][:].rearrange("(k p) d -> p k d", p=128)
            z1_src = z_w[1][0].rearrange("(k p) d -> p k d", p=128)
            mn1_src = w_mean[1][:].rearrange("(k p) d -> p k d", p=128)
            for k in range(NK[1]):
                hwd.dma_start(sg1[:, k, :], sg1_src[:, k, :])
                nc.scalar.activation(sg1[:, k, :], sg1[:, k, :], AF.Exp)
            for k in range(NK[1]):
                hwd.dma_start(zt1[:, k, :], z1_src[:, k, :])
            for k in range(NK[1]):
                hwd.dma_start(mn1[:, k, :], mn1_src[:, k, :])
                nc.vector.tensor_mul(zt1[:, k, :], zt1[:, k, :], sg1[:, k, :])
                nc.vector.tensor_add(wt1[:, k, :], zt1[:, k, :], mn1[:, k, :])
            w_tiles[(1, 0)] = wt1
            emit_bias_prep(1, 0)

            emit_l01_mms(0, 2)
            emit_l01_mms(1, 0, korder=True)

            # ---- L0 s3 ----
            emit_wprep(0, 3, w0p, half_dma=True)
            emit_l01_mms(0, 3)

            # ---- L1 s1 ----
            emit_wprep(1, 1, w1p, half_dma=True)
            emit_l01_mms(1, 1)

            # ---- layer-2 consts + pair 0 preps (emitted before L2 MMs) ----
            sg2 = cpool.tile([128, NK[2], DIMS[3]], F32, tag="sigma2")
            mn2 = cpool.tile([128, NK[2], DIMS[3]], F32, tag="mean2")
            sigma[2], mean[2] = sg2, mn2
            hwd.dma_start(sg2[:], w_ls[2][:].rearrange("(k p) d -> p k d", p=128))
            nc.scalar.activation(sg2[:], sg2[:], AF.Exp)
            hwd.dma_start(mn2[:], w_mean[2][:].rearrange("(k p) d -> p k d", p=128))
            emit_wprep(2, 0, w2p)
            emit_wprep(2, 1, w2p)
            emit_bias_prep(2, 0)
            emit_bias_prep(2, 1)
            emit_l2_pair(0)

            # ---- L1 s2, s3; L2 pair 1 preps run ahead on DVE ----
            emit_wprep(1, 2, w1p, half_dma=True)
            emit_l01_mms(1, 2)
            emit_wprep(1, 3, w1p, half_dma=True)
            emit_wprep(2, 2, w2p)
            emit_wprep(2, 3, w2p)
            emit_l01_mms(1, 3)
            emit_l2_pair(1, tail=True)

    nc.compile()
    return nc


def _get_nc():
    if "nc" not in _CACHE:
        _CACHE["nc"] = _build_nc()
    return _CACHE["nc"]


def _pack_bias(inp, s0):
    """Pack z_b / b_mean / b_log_std for samples [s0, s0+SL) into the
    [128, BP_W] SBUF-layout tensor (pure layout work)."""
    bp = np.zeros((128, BP_W), np.float32)
    for li in (0, 1):
        c = BC[li]
        bp[:, BLS[li] : BLS[li] + c] = inp[f"b_log_std_{li}"].reshape(c, 128).T
        bp[:, BMN[li] : BMN[li] + c] = inp[f"b_mean_{li}"].reshape(c, 128).T
        zb = inp[f"z_b_{li}"][s0 : s0 + SL, 0, :]
        for s_ in range(SL):
            bp[:, BZB[li] + c * s_ : BZB[li] + c * (s_ + 1)] = zb[s_].reshape(c, 128).T
    # layer 2: 64 partitions, duplicated for the column-strip sample pairing
    for half in (slice(0, 64), slice(64, 128)):
        bp[half, BLS[2]] = inp["b_log_std_2"]
        bp[half, BMN[2]] = inp["b_mean_2"]
    zb2 = inp["z_b_2"][s0 : s0 + SL, 0, :]
    bp[0:64, BZB[2]] = zb2[0]
    bp[64:128, BZB[2]] = zb2[1]
    bp[0:64, BZB[2] + 1] = zb2[2]
    bp[64:128, BZB[2] + 1] = zb2[3]
    return bp


def kernel(**inputs) -> np.ndarray:
    global LAST_RESULT
    nc = _get_nc()
    inp = {k: np.asarray(v, dtype=np.float32) for k, v in inputs.items()}

    xT = np.ascontiguousarray(inp["x"].T)
    in_maps = []
    for c in range(NCORES):
        sl = slice(c * SL, (c + 1) * SL)
        m = {"xT": xT, "biaspack": _pack_bias(inp, c * SL)}
        for li in range(3):
            m[f"w_mean_{li}"] = inp[f"w_mean_{li}"]
            m[f"w_log_std_{li}"] = inp[f"w_log_std_{li}"]
            m[f"z_w_{li}"] = np.ascontiguousarray(inp[f"z_w_{li}"][sl])
        in_maps.append(m)

    res = bass_utils.run_bass_kernel_spmd(
        nc, in_maps, core_ids=list(range(NCORES)), **RUN_KWARGS
    )
    LAST_RESULT = res
    full = np.concatenate([res.results[c]["out"] for c in range(NCORES)], axis=0)
    return np.ascontiguousarray(full.transpose(0, 2, 1)).astype(np.float32)
